# revision 9
# baseline (speedup 1.0000x reference)
"""Causal multi-head attention (B=16, T=1024, E=1024, H=16, Dh=64) on 8 TRN2
NeuronCores.

Sharding: data-parallel over batch -- 2 batch elements per core, weights
replicated, no collectives. Host pre-transposes x and packs weights; each core
runs an identical Bass/Tile program on its shard.

Per-core dataflow (all in "transposed" orientation so no on-chip transposes
are ever needed):
  x^T [E,T] (host)   --matmul-->  Q^T,K^T [Dh,T] per head (head-pairs packed
                                  into 128 partitions; 1/sqrt(Dh) folded into
                                  the Q PSUM->SBUF copy)
                     --matmul-->  V [T,Dh] per head (+ ones column)
  S^T[tk,tq] = (K^T tile).T @ Q^T  per key-tile, causal tiles skipped
  P^T = exp(S^T) on ScalarE (scores are O(1): no max subtraction needed);
        diagonal tiles masked by 0/1 multiply
  O'^T[65,tq] += (V'|1).T @ P^T   -- row 64 accumulates the softmax denom
  Y^T = O'^T[0:64] * bcast(1/denom)
  out[t,E] = Y^T.T @ Wo + bo
"""
import numpy as np
import ml_dtypes

import concourse.bass as bass
import concourse.mybir as mybir
import concourse.tile as tile
from concourse import bacc
from concourse.bass_utils import run_bass_kernel_spmd

B, T, E = 16, 1024, 1024
H, Dh = 16, 64
NCORES = 8
BL = B // NCORES          # batches per core
P = 128                   # partitions
ET = E // P               # 8 tiles along E / token / hd dims
HP = H // 2               # 8 head-pairs
BF = mybir.dt.bfloat16
F32 = mybir.dt.float32
AF = mybir.ActivationFunctionType

_CACHE = {}


def _pieces(i):
    """Column pieces of [128*i, 1024) that do not cross the 512 PSUM-bank
    boundary."""
    if i < 4:
        return [(128 * i, 512), (512, 1024)]
    return [(128 * i, 1024)]


def _build(dbg=False):
    nc = bacc.Bacc("TRN2", target_bir_lowering=False, debug=False,
                   num_devices=NCORES)

    dbg_out = {}
    if dbg:
        for name, shape, dt in [
            ("d_qT", [P, HP, T], BF), ("d_kT", [P, HP, T], BF),
            ("d_v", [P, ET, H, Dh + 1], BF), ("d_pt", [ET, P, T], BF),
            ("d_s0", [P, T], F32), ("d_op", [P, T], F32),
            ("d_r1", [1, T], F32), ("d_rb", [Dh, T], F32),
            ("d_yT", [P, HP, T], BF),
        ]:
            dbg_out[name] = nc.dram_tensor(name, shape, dt,
                                           kind="ExternalOutput").ap()

    xT = nc.dram_tensor("xT", [BL, E, T], BF, kind="ExternalInput").ap()
    wq = nc.dram_tensor("wq", [E, H * Dh], BF, kind="ExternalInput").ap()
    wk = nc.dram_tensor("wk", [E, H * Dh], BF, kind="ExternalInput").ap()
    wv = nc.dram_tensor("wv", [E, H * Dh], BF, kind="ExternalInput").ap()
    wo = nc.dram_tensor("wo", [H * Dh, E], BF, kind="ExternalInput").ap()
    borep = nc.dram_tensor("borep", [P, E], F32, kind="ExternalInput").ap()
    mask01 = nc.dram_tensor("mask01", [P, P], BF, kind="ExternalInput").ap()
    out = nc.dram_tensor("out", [BL, T, E], F32, kind="ExternalOutput").ap()

    with tile.TileContext(nc) as tc:
        with (
            tc.tile_pool(name="consts", bufs=1) as cpool,
            tc.tile_pool(name="xp", bufs=1) as xpool,
            tc.tile_pool(name="qk", bufs=1) as qkpool,
            tc.tile_pool(name="vy", bufs=1) as vypool,
            tc.tile_pool(name="pt", bufs=3) as ptpool,
            tc.tile_pool(name="sm", bufs=2) as spool,
            tc.tile_pool(name="ob", bufs=3) as opool,
            tc.tile_pool(name="ps", bufs=4, space="PSUM") as ps,
        ):
            wq_sb = cpool.tile([P, ET, H * Dh], BF, tag="wq")
            wk_sb = cpool.tile([P, ET, H * Dh], BF, tag="wk")
            wv_sb = cpool.tile([P, ET, H * Dh], BF, tag="wv")
            wo_sb = cpool.tile([P, ET, E], BF, tag="wo")
            nc.sync.dma_start(wq_sb[:], wq.rearrange("(n p) c -> p n c", p=P))
            nc.sync.dma_start(wk_sb[:], wk.rearrange("(n p) c -> p n c", p=P))
            nc.sync.dma_start(wv_sb[:], wv.rearrange("(n p) c -> p n c", p=P))
            nc.sync.dma_start(wo_sb[:], wo.rearrange("(n p) c -> p n c", p=P))
            borep_sb = cpool.tile([P, E], F32, tag="bo")
            nc.sync.dma_start(borep_sb[:], borep)
            mask_sb = cpool.tile([P, P], BF, tag="mask")
            nc.sync.dma_start(mask_sb[:], mask01)

            for b in range(BL):
                xT_sb = xpool.tile([P, ET, T], BF, tag="xT")
                nc.sync.dma_start(
                    xT_sb[:], xT[b].rearrange("(n p) c -> p n c", p=P))

                # ---- V projection: V'[tok, head, 65] (col 64 = ones) ----
                v_sb = vypool.tile([P, ET, H, Dh + 1], BF, tag="v")
                nc.vector.memset(v_sb[:, :, :, Dh], 1.0)
                for t in range(ET):
                    vp = ps.tile([P, 1024], F32, tag="ps")
                    for n2 in range(2):
                        cs = slice(512 * n2, 512 * (n2 + 1))
                        for i in range(ET):
                            nc.tensor.matmul(
                                vp[:, cs],
                                lhsT=xT_sb[:, i, 128 * t:128 * (t + 1)],
                                rhs=wv_sb[:, i, cs],
                                start=(i == 0), stop=(i == ET - 1),
                            )
                        nc.scalar.activation(
                            v_sb[:, t, 8 * n2:8 * (n2 + 1), 0:Dh],
                            vp[:, cs].rearrange("p (h d) -> p h d", d=Dh),
                            AF.Copy,
                        )

                # ---- Q^T / K^T projections (head-pair packed) ----
                qT = qkpool.tile([P, HP, T], BF, tag="q")
                kT = qkpool.tile([P, HP, T], BF, tag="k")
                for hp in range(HP):
                    qp = ps.tile([P, 1024], F32, tag="ps")
                    kp = ps.tile([P, 1024], F32, tag="ps")
                    for n2 in range(2):
                        cs = slice(512 * n2, 512 * (n2 + 1))
                        for i in range(ET):
                            nc.tensor.matmul(
                                qp[:, cs],
                                lhsT=wq_sb[:, i, 128 * hp:128 * (hp + 1)],
                                rhs=xT_sb[:, i, cs],
                                start=(i == 0), stop=(i == ET - 1),
                            )
                        for i in range(ET):
                            nc.tensor.matmul(
                                kp[:, cs],
                                lhsT=wk_sb[:, i, 128 * hp:128 * (hp + 1)],
                                rhs=xT_sb[:, i, cs],
                                start=(i == 0), stop=(i == ET - 1),
                            )
                    # PSUM->SBUF casts; attention scale folded into Q
                    nc.scalar.activation(qT[:, hp, :], qp[:], AF.Copy,
                                         scale=1.0 / float(np.sqrt(Dh)))
                    nc.scalar.activation(kT[:, hp, :], kp[:], AF.Copy)

                if dbg and b == 0:
                    nc.sync.dma_start(dbg_out["d_qT"], qT[:])
                    nc.sync.dma_start(dbg_out["d_kT"], kT[:])
                    nc.sync.dma_start(dbg_out["d_v"], v_sb[:])

                # ---- attention per head ----
                yT = vypool.tile([P, HP, T], BF, tag="y")
                for h in range(H):
                    hp, po = h // 2, Dh * (h % 2)
                    op_ = ps.tile([P, 1024], F32, tag="ps")
                    for i in range(ET):
                        sp_ = ps.tile([P, 1024], F32, tag="ps")
                        for (a0, a1) in _pieces(i):
                            nc.tensor.matmul(
                                sp_[:, a0:a1],
                                lhsT=kT[po:po + Dh, hp, 128 * i:128 * (i + 1)],
                                rhs=qT[po:po + Dh, hp, a0:a1],
                                start=True, stop=True,
                            )
                        pt = ptpool.tile([P, 1024], BF, tag="pt")
                        for (a0, a1) in _pieces(i):
                            nc.scalar.activation(pt[:, a0:a1], sp_[:, a0:a1],
                                                 AF.Exp)
                        # zero the below-diagonal half of the diagonal tile
                        ds_ = slice(128 * i, 128 * (i + 1))
                        nc.vector.tensor_mul(pt[:, ds_], pt[:, ds_], mask_sb[:])
                        if dbg and b == 0 and h == 0:
                            if i == 0:
                                s0c = spool.tile([P, T], F32, tag="dbg_s0")
                                nc.scalar.activation(s0c[:], sp_[:], AF.Copy)
                                nc.sync.dma_start(dbg_out["d_s0"], s0c[:])
                            nc.sync.dma_start(dbg_out["d_pt"][i], pt[:])
                        for (a0, a1) in _pieces(i):
                            nc.tensor.matmul(
                                op_[0:Dh + 1, a0:a1],
                                lhsT=v_sb[:, i, h, :],
                                rhs=pt[:, a0:a1],
                                start=(i == 0), stop=(i == ET - 1),
                                skip_group_check=True,
                            )
                    # 1/denom = exp(-ln(denom)) on ScalarE (the fast custom
                    # DVE reciprocal mis-executes on HW in this runtime path)
                    ln1 = spool.tile([1, T], F32, tag="ln1")
                    nc.scalar.activation(ln1[:], op_[Dh:Dh + 1, :], AF.Ln)
                    r1 = spool.tile([1, T], F32, tag="r1")
                    nc.scalar.activation(r1[:], ln1[:], AF.Exp, scale=-1.0)
                    rb = spool.tile([Dh, T], F32, tag="rb")
                    nc.gpsimd.partition_broadcast(rb[:], r1[:])
                    if dbg and b == 0 and h == 0:
                        opc = spool.tile([P, T], F32, tag="dbg_op")
                        nc.scalar.activation(opc[:], op_[:], AF.Copy)
                        nc.sync.dma_start(dbg_out["d_op"], opc[:])
                        nc.sync.dma_start(dbg_out["d_r1"], r1[:])
                        nc.sync.dma_start(dbg_out["d_rb"], rb[:])
                    nc.vector.tensor_mul(yT[po:po + Dh, hp, :], op_[0:Dh, :],
                                         rb[:])

                if dbg and b == 0:
                    nc.sync.dma_start(dbg_out["d_yT"], yT[:])

                # ---- output projection + bias ----
                for t in range(ET):
                    o2 = ps.tile([P, 1024], F32, tag="ps")
                    for n2 in range(2):
                        cs = slice(512 * n2, 512 * (n2 + 1))
                        for j in range(ET):
                            nc.tensor.matmul(
                                o2[:, cs],
                                lhsT=yT[:, j, 128 * t:128 * (t + 1)],
                                rhs=wo_sb[:, j, cs],
                                start=(j == 0), stop=(j == ET - 1),
                            )
                    ob = opool.tile([P, E], F32, tag="ob")
                    nc.vector.tensor_add(ob[:], o2[:], borep_sb[:])
                    nc.sync.dma_start(out[b, 128 * t:128 * (t + 1), :], ob[:])

    nc.compile()
    return nc


def _get_nc():
    if "nc" not in _CACHE:
        _CACHE["nc"] = _build()
    return _CACHE["nc"]


def _prep_in_maps(x, Wq, Wk, Wv, Wo, bo):
    bf16 = ml_dtypes.bfloat16
    # [B,T,E] -> [B,E,T] transposed activations
    xT = np.ascontiguousarray(x.transpose(0, 2, 1)).astype(bf16)
    # [H,E,Dh] -> [E, H*Dh] (heads side by side so a 128-col slice = 2 heads)
    wq_pk = np.ascontiguousarray(Wq.transpose(1, 0, 2).reshape(E, H * Dh)).astype(bf16)
    wk_pk = np.ascontiguousarray(Wk.transpose(1, 0, 2).reshape(E, H * Dh)).astype(bf16)
    wv_pk = np.ascontiguousarray(Wv.transpose(1, 0, 2).reshape(E, H * Dh)).astype(bf16)
    wo_b = np.ascontiguousarray(Wo).astype(bf16)
    borep = np.ascontiguousarray(
        np.broadcast_to(bo.astype(np.float32), (P, E)))
    ii, jj = np.mgrid[0:P, 0:P]
    mask01 = (jj >= ii).astype(bf16)  # S^T[tk,tq]: keep tq >= tk

    in_maps = []
    for c in range(NCORES):
        in_maps.append({
            "xT": xT[BL * c:BL * (c + 1)],
            "wq": wq_pk, "wk": wk_pk, "wv": wv_pk, "wo": wo_b,
            "borep": borep, "mask01": mask01,
        })
    return in_maps


def run(inputs, trace=False):
    """Returns (full_output [B,T,E] fp32, BassKernelResults)."""
    nc = _get_nc()
    in_maps = _prep_in_maps(**inputs)
    res = run_bass_kernel_spmd(nc, in_maps, core_ids=list(range(NCORES)),
                               trace=trace)
    out = np.concatenate([res.results[c]["out"] for c in range(NCORES)],
                         axis=0)
    return out, res


def kernel(x, Wq, Wk, Wv, Wo, bo):
    out, _ = run(dict(x=x, Wq=Wq, Wk=Wk, Wv=Wv, Wo=Wo, bo=bo))
    return out
